# revision 17
# baseline (speedup 1.0000x reference)
"""Categorical cross-entropy loss kernel for Trainium2 (8 NeuronCores).

Computes: out = [-sum(input * log(target + 1e-8)) / B] for input/target of
shape [B=262144, C=128] float32.

Strategy (data-parallel, memory-bound streaming reduction):
  - Shard both tensors along batch across 8 cores (32768 rows each).
  - Each core views its [32768, 128] shard as [128 partitions, 32768 free]
    (partition p owns 256 contiguous rows -> contiguous 128 KiB per
    partition), streams it in 8 uniform chunks of [128, 4096] (2 MiB DMAs,
    16 KiB contiguous per partition = SDMA line rate).  Steady state is at
    the HBM-per-core limit (~358 GB/s spec; measured 96-97 us/pass for the
    33.5 MB/core of traffic), DMA-bound with compute fully hidden.
  - Per chunk: ACT computes log(target + eps) out of place, then DVE
    multiplies by input and reduces over the free axis into acc[:, j].
  - DMA rings (split_rings=True): target chunks on the sync HWDGE ring,
    input chunks on the scalar HWDGE ring; the SDMA engines round-robin
    both descriptor queues, covering inter-transfer gaps (~0.5-1 us/pass
    measured vs single-ring).  A third stream via gpsimd SWDGE measured
    8 us/pass SLOWER (SWDGE fixed costs) -- don't.
  - Tail (tail_split=4): both rings finish together, so BOTH final chunks
    land as descending pieces [1280,1280,1024,512] on their own rings,
    each piece chased by sliced ACT (log) then DVE (mult+reduce).  The
    serial drain after the last DMA byte is one 512-wide slice (~1 us)
    instead of a whole 4096 chunk (~6 us).  Same total bytes and +3
    transfers/ring; A/B slope measurement showed no steady-state cost
    (96.6 us/pass with and without).
  - Per-core output: [128, 11] partial sums; host sums in float64, scales
    by -1/B.
"""

import numpy as np

import concourse.bass as bass
import concourse.tile as tile
from concourse import bacc, mybir
from concourse.bass_utils import run_bass_kernel_spmd

B, C = 262144, 128
NCORES = 8
ROWS = B // NCORES          # 32768 rows per core
P = 128                     # SBUF partitions
FREE = ROWS * C // P        # 32768 f32 per partition
EPS = 1e-8

_NC_CACHE = None


# Uniform full-width chunks: every DMA is [128, 4096] f32 = 2 MiB with
# 16 KiB contiguous per partition -- max SDMA line rate.  A tapered DMA
# tail (4096..128) was measured 10 us/pass SLOWER in steady state: the
# small trailing chunks pay the sub-1MiB DMA efficiency cliff on every
# pass.  tail_split (below) is the cheap version of the same idea: only
# the final tgt/inp chunks are split, into >=0.25 MiB pieces (still
# efficient); A/B slope measurement showed no steady-state cost.
CH_SCHEDULE = [4096] * 8
assert sum(CH_SCHEDULE) == FREE


def build_nc(repeat: int = 1, ch_schedule=None, io_bufs: int = 3,
             scratch_bufs: int = 3, inplace_mult: bool = False,
             alt_dma: bool = False, split_rings: bool = True,
             compute: str = "full", act_oop: bool = True,
             lean_preamble: bool = True, warmup_dma: bool = False,
             layout: str = "strided", tail_split: int = 4) -> bass.Bass:
    if ch_schedule is None:
        ch_schedule = CH_SCHEDULE
    assert sum(ch_schedule) == FREE
    nch = len(ch_schedule)
    offs = [0]
    for c in ch_schedule:
        offs.append(offs[-1] + c)
    max_ch = max(ch_schedule)
    if layout == "seq":
        # chunk j = contiguous DRAM range [j*ch*P*4, ...): partition p owns
        # the p-th (ch*4)-byte span of it.  Requires uniform chunks whose
        # row count (=ch) splits into whole rows per partition (ch%P==0).
        assert len(set(ch_schedule)) == 1 and ch_schedule[0] % P == 0
    assert tail_split == 1 or (layout == "strided" and compute == "full"
                               and act_oop and tail_split >= 2
                               and ch_schedule[-1] % tail_split == 0)
    nc = bacc.Bacc("TRN2", target_bir_lowering=False, debug=False,
                   num_devices=NCORES)
    if lean_preamble:
        # Bass.__init__ memsets 4 const APs (0.0/1.0 f32, 1.0 bf16, 127 u8)
        # on gpsimd before the init barrier; nothing in this kernel reads
        # them (the eps bias is our own tile), so drop the serial memsets.
        # The barrier instructions stay -- removal only unwrites tensors
        # that have no readers, so it cannot introduce a race.
        bb = nc.cur_bb.bb
        bb.instructions = [
            i for i in bb.instructions
            if not (isinstance(i, mybir.InstMemset)
                    and i.outs and "const-" in str(i.outs[0]))
        ]
    inp = nc.dram_tensor("input", [ROWS, C], mybir.dt.float32,
                         kind="ExternalInput").ap()
    tgt = nc.dram_tensor("target", [ROWS, C], mybir.dt.float32,
                         kind="ExternalInput").ap()
    n_out = nch - 1 + tail_split
    out = nc.dram_tensor("out", [P, n_out], mybir.dt.float32,
                         kind="ExternalOutput").ap()

    if layout == "seq":
        inp_v3 = inp.rearrange("(j p n) c -> p j (n c)", p=P, j=nch)
        tgt_v3 = tgt.rearrange("(j p n) c -> p j (n c)", p=P, j=nch)
        inp_src = lambda j, ch: inp_v3[:, j]
        tgt_src = lambda j, ch: tgt_v3[:, j]
    else:
        inp_v = inp.rearrange("(p n) c -> p (n c)", p=P)
        tgt_v = tgt.rearrange("(p n) c -> p (n c)", p=P)
        inp_src = lambda j, ch: inp_v[:, offs[j]:offs[j] + ch]
        tgt_src = lambda j, ch: tgt_v[:, offs[j]:offs[j] + ch]

    with tile.TileContext(nc) as tc:
        with (
            tc.tile_pool(name="eps", bufs=1) as eps_pool,
            tc.tile_pool(name="io", bufs=io_bufs) as io_pool,
            tc.tile_pool(name="scratch", bufs=scratch_bufs) as scratch_pool,
            tc.tile_pool(name="acc", bufs=1) as acc_pool,
        ):
            # EPS bias for the ACT Ln; Tile tracks the memset->ACT dep so
            # it overlaps the first DMAs (no extra all-engine barrier)
            if compute != "none":
                eps_t = eps_pool.tile([P, 1], mybir.dt.float32)
                nc.gpsimd.memset(eps_t[:], EPS)
            if warmup_dma:
                wt = eps_pool.tile([P, 1], mybir.dt.float32, tag="warm")
                nc.sync.dma_start(wt[:], inp_src(0, max_ch)[:, 0:1])
                nc.vector.tensor_copy(wt[:], wt[:])  # keep a reader

            acc = None
            if compute == "full":
                acc = acc_pool.tile([P, n_out], mybir.dt.float32)
            last_tt = None
            for it in range(nch * repeat):
                j = it % nch
                ch = ch_schedule[j]
                if alt_dma == "cross":
                    # both HWDGE rings busy every chunk, each carrying one
                    # tensor, swapping per chunk so neither ring owns a
                    # tensor's full stream
                    dma = nc.sync if it % 2 == 0 else nc.scalar
                elif alt_dma == "crossg":
                    # like cross, but the second issue stream is gpsimd
                    # SWDGE (otherwise idle) instead of the ACT/scalar
                    # queue, which also carries the Ln activations
                    dma = nc.sync if it % 2 == 0 else nc.gpsimd
                elif alt_dma == "tri":
                    # three issue streams (2 HWDGE rings + gpsimd SWDGE):
                    # probes whether per-transfer issue gaps on a ring cost
                    # steady-state bandwidth
                    dma = (nc.sync, nc.scalar, nc.gpsimd)[it % 3]
                else:
                    dma = nc.scalar if (alt_dma and it % 2) else nc.sync
                if alt_dma == "cross":
                    dma_inp = nc.scalar if it % 2 == 0 else nc.sync
                elif alt_dma == "crossg":
                    dma_inp = nc.gpsimd if it % 2 == 0 else nc.sync
                elif alt_dma == "tri":
                    dma_inp = (nc.scalar, nc.gpsimd, nc.sync)[it % 3]
                elif split_rings == "gpsimd":
                    dma_inp = nc.gpsimd
                elif split_rings:
                    dma_inp = nc.scalar
                else:
                    dma_inp = dma
                if tail_split > 1 and j == nch - 1:
                    # Tail drain shortener: the pass's LAST tgt AND inp
                    # chunks each land as tail_split small transfers (on
                    # their own rings under split_rings, where both rings
                    # finish together), each slice chased by ACT then DVE,
                    # so the serial drain after the final DMA byte is one
                    # sub-slice (ch/tail_split) instead of a whole chunk.
                    # Same total bytes; the extra transfers sit at the
                    # stream end.  DMA-issue instructions for the scalar
                    # ring are emitted BEFORE the Ln slices so they are not
                    # queued behind ACT compute on the shared sequencer.
                    # Descending pieces: same transfer count as equal
                    # pieces, but the FINAL piece (which alone sets the
                    # serial drain) is ch/8 instead of ch/4.
                    if tail_split == 4 and ch % 16 == 0:
                        psched = [ch * 5 // 16, ch * 5 // 16,
                                  ch * 4 // 16, ch * 2 // 16]
                    else:
                        psched = [ch // tail_split] * tail_split
                    poffs = [offs[j]]
                    for c in psched:
                        poffs.append(poffs[-1] + c)
                    prod = scratch_pool.tile([P, max_ch], mybir.dt.float32)
                    tqs, tps = [], []
                    for s in range(tail_split):
                        tq = io_pool.tile([P, psched[s]], mybir.dt.float32,
                                          tag=f"tailq{s}", bufs=1,
                                          name=f"tq{s}")
                        dma.dma_start(tq[:], tgt_v[:, poffs[s]:poffs[s + 1]])
                        tqs.append(tq)
                    for s in range(tail_split):
                        tp = io_pool.tile([P, psched[s]], mybir.dt.float32,
                                          tag=f"tail{s}", bufs=1,
                                          name=f"tp{s}")
                        dma_inp.dma_start(tp[:],
                                          inp_v[:, poffs[s]:poffs[s + 1]])
                        tps.append(tp)
                    base = poffs[0]
                    for s in range(tail_split):
                        sl = slice(poffs[s] - base, poffs[s + 1] - base)
                        nc.scalar.activation(prod[:, sl], tqs[s][:],
                                             mybir.ActivationFunctionType.Ln,
                                             bias=eps_t[:])
                        nc.vector.tensor_tensor(prod[:, sl], tps[s][:],
                                                prod[:, sl],
                                                mybir.AluOpType.mult)
                        nc.vector.tensor_reduce(acc[:, j + s:j + s + 1],
                                                prod[:, sl],
                                                mybir.AxisListType.X,
                                                mybir.AluOpType.add)
                    last_tt = tqs[-1]
                    continue
                # target first: ACT only needs tgt, so it can start while
                # input is still in flight
                tt = io_pool.tile([P, max_ch], mybir.dt.float32, tag="tgt")
                dma.dma_start(tt[:, :ch], tgt_src(j, ch))
                ti = io_pool.tile([P, max_ch], mybir.dt.float32, tag="inp")
                dma_inp.dma_start(ti[:, :ch], inp_src(j, ch))
                last_tt = tt
                if compute == "none":
                    continue
                if act_oop:
                    # log into scratch: tt's buffer frees right after ACT
                    # reads it, giving tgt DMAs one more stage of lead time
                    prod = scratch_pool.tile([P, max_ch], mybir.dt.float32)
                    nc.scalar.activation(prod[:, :ch], tt[:, :ch],
                                         mybir.ActivationFunctionType.Ln,
                                         bias=eps_t[:])
                    if compute == "act":
                        continue
                    nc.vector.tensor_tensor(prod[:, :ch], ti[:, :ch],
                                            prod[:, :ch],
                                            mybir.AluOpType.mult)
                    nc.vector.tensor_reduce(acc[:, j:j + 1], prod[:, :ch],
                                            mybir.AxisListType.X,
                                            mybir.AluOpType.add)
                    continue
                # tt = log(tt + EPS), in place on the ACT engine
                nc.scalar.activation(tt[:, :ch], tt[:, :ch],
                                     mybir.ActivationFunctionType.Ln,
                                     bias=eps_t[:])
                if compute == "act":
                    continue
                # acc[:, j] = sum_free(ti * tt)
                # (TensorTensorReduce would fuse these, but it crashes the
                # device on this runtime build -- use 2 DVE ops instead)
                if inplace_mult:
                    prod = ti
                else:
                    prod = scratch_pool.tile([P, max_ch], mybir.dt.float32)
                nc.vector.tensor_tensor(prod[:, :ch], ti[:, :ch], tt[:, :ch],
                                        mybir.AluOpType.mult)
                nc.vector.tensor_reduce(acc[:, j:j + 1], prod[:, :ch],
                                        mybir.AxisListType.X,
                                        mybir.AluOpType.add)
            if compute == "full":
                nc.sync.dma_start(out[:], acc[:])
            else:  # timing probes: output is garbage, deps only on last tile
                nc.sync.dma_start(out[:], last_tt[:, :n_out])
    nc.compile()
    return nc


def shard_inputs(inp: np.ndarray, tgt: np.ndarray) -> list[dict]:
    return [
        {
            "input": np.ascontiguousarray(inp[i * ROWS:(i + 1) * ROWS]),
            "target": np.ascontiguousarray(tgt[i * ROWS:(i + 1) * ROWS]),
        }
        for i in range(NCORES)
    ]


def combine(results: list[dict]) -> np.ndarray:
    total = 0.0
    for r in results:
        total += float(np.sum(np.asarray(r["out"], dtype=np.float64)))
    return np.array([-total / B], dtype=np.float32)


def kernel(**inputs: np.ndarray) -> np.ndarray:
    global _NC_CACHE
    inp = np.ascontiguousarray(np.asarray(inputs["input"], dtype=np.float32))
    tgt = np.ascontiguousarray(np.asarray(inputs["target"], dtype=np.float32))
    assert inp.shape == (B, C) and tgt.shape == (B, C)

    if _NC_CACHE is None:
        _NC_CACHE = build_nc()
    nc = _NC_CACHE

    res = run_bass_kernel_spmd(nc, shard_inputs(inp, tgt),
                               list(range(NCORES)))
    return combine(res.results)



# revision 20
# speedup vs baseline: 1.0211x; 1.0211x over previous
"""Categorical cross-entropy loss kernel for Trainium2 (8 NeuronCores).

Computes: out = [-sum(input * log(target + 1e-8)) / B] for input/target of
shape [B=262144, C=128] float32.

Strategy (data-parallel, memory-bound streaming reduction):
  - Shard both tensors along batch across 8 cores (32768 rows each).
  - Each core views its [32768, 128] shard as [128 partitions, 32768 free]
    (partition p owns 256 contiguous rows -> contiguous 128 KiB per
    partition), streams it in 8 uniform chunks of [128, 4096] (2 MiB DMAs,
    16 KiB contiguous per partition = SDMA line rate).  Steady state is at
    the HBM-per-core limit (~358 GB/s spec; measured 96-97 us/pass for the
    33.5 MB/core of traffic), DMA-bound with compute fully hidden.
  - Per chunk: ACT computes log(target + eps) out of place, then DVE
    multiplies by input and reduces over the free axis into acc[:, j].
  - DMA rings (split_rings=True): target chunks on the sync HWDGE ring,
    input chunks on the scalar HWDGE ring; the SDMA engines round-robin
    both descriptor queues, covering inter-transfer gaps (~0.5-1 us/pass
    measured vs single-ring).  A third stream via gpsimd SWDGE measured
    8 us/pass SLOWER (SWDGE fixed costs) -- don't.
  - Tail (tail_split=4): both rings finish together, so BOTH final chunks
    land as descending pieces [1280,1280,1024,512] on their own rings,
    each piece chased by sliced ACT (log) then DVE (mult+reduce).  The
    serial drain after the last DMA byte is one 512-wide slice (~1 us)
    instead of a whole 4096 chunk (~6 us).  Same total bytes and +3
    transfers/ring; A/B slope measurement showed no steady-state cost
    (96.6 us/pass with and without).
  - Per-core output: [128, 11] partial sums; host sums in float64, scales
    by -1/B.
  - Host side: under axon, one cached jitted SPMD executable serves all
    kernel() calls (run_bass_kernel_spmd would re-trace, re-lower and --
    without a persistent jax cache -- re-run neuronxcc on every call;
    measured 3.5 s/call cached vs 20+ s uncached).  Native environments
    fall back to run_bass_kernel_spmd.
"""

import numpy as np

import concourse.bass as bass
import concourse.tile as tile
from concourse import bacc, mybir
from concourse.bass_utils import run_bass_kernel_spmd

B, C = 262144, 128
NCORES = 8
ROWS = B // NCORES          # 32768 rows per core
P = 128                     # SBUF partitions
FREE = ROWS * C // P        # 32768 f32 per partition
EPS = 1e-8

_NC_CACHE = None


# Uniform full-width chunks: every DMA is [128, 4096] f32 = 2 MiB with
# 16 KiB contiguous per partition -- max SDMA line rate.  A tapered DMA
# tail (4096..128) was measured 10 us/pass SLOWER in steady state: the
# small trailing chunks pay the sub-1MiB DMA efficiency cliff on every
# pass.  tail_split (below) is the cheap version of the same idea: only
# the final tgt/inp chunks are split, into >=0.25 MiB pieces (still
# efficient); A/B slope measurement showed no steady-state cost.
CH_SCHEDULE = [4096] * 8
assert sum(CH_SCHEDULE) == FREE


def build_nc(repeat: int = 1, ch_schedule=None, io_bufs: int = 3,
             scratch_bufs: int = 3, inplace_mult: bool = False,
             alt_dma: bool = False, split_rings: bool = True,
             compute: str = "full", act_oop: bool = True,
             lean_preamble: bool = True, warmup_dma: bool = False,
             layout: str = "strided", tail_split: int = 4) -> bass.Bass:
    if ch_schedule is None:
        ch_schedule = CH_SCHEDULE
    assert sum(ch_schedule) == FREE
    nch = len(ch_schedule)
    offs = [0]
    for c in ch_schedule:
        offs.append(offs[-1] + c)
    max_ch = max(ch_schedule)
    if layout == "seq":
        # chunk j = contiguous DRAM range [j*ch*P*4, ...): partition p owns
        # the p-th (ch*4)-byte span of it.  Requires uniform chunks whose
        # row count (=ch) splits into whole rows per partition (ch%P==0).
        assert len(set(ch_schedule)) == 1 and ch_schedule[0] % P == 0
    assert tail_split == 1 or (layout == "strided" and compute == "full"
                               and act_oop and tail_split >= 2
                               and ch_schedule[-1] % tail_split == 0)
    nc = bacc.Bacc("TRN2", target_bir_lowering=False, debug=False,
                   num_devices=NCORES)
    if lean_preamble:
        # Bass.__init__ memsets 4 const APs (0.0/1.0 f32, 1.0 bf16, 127 u8)
        # on gpsimd before the init barrier; nothing in this kernel reads
        # them (the eps bias is our own tile), so drop the serial memsets.
        # The barrier instructions stay -- removal only unwrites tensors
        # that have no readers, so it cannot introduce a race.
        bb = nc.cur_bb.bb
        bb.instructions = [
            i for i in bb.instructions
            if not (isinstance(i, mybir.InstMemset)
                    and i.outs and "const-" in str(i.outs[0]))
        ]
    inp = nc.dram_tensor("input", [ROWS, C], mybir.dt.float32,
                         kind="ExternalInput").ap()
    tgt = nc.dram_tensor("target", [ROWS, C], mybir.dt.float32,
                         kind="ExternalInput").ap()
    n_out = nch - 1 + tail_split
    out = nc.dram_tensor("out", [P, n_out], mybir.dt.float32,
                         kind="ExternalOutput").ap()

    if layout == "seq":
        inp_v3 = inp.rearrange("(j p n) c -> p j (n c)", p=P, j=nch)
        tgt_v3 = tgt.rearrange("(j p n) c -> p j (n c)", p=P, j=nch)
        inp_src = lambda j, ch: inp_v3[:, j]
        tgt_src = lambda j, ch: tgt_v3[:, j]
    else:
        inp_v = inp.rearrange("(p n) c -> p (n c)", p=P)
        tgt_v = tgt.rearrange("(p n) c -> p (n c)", p=P)
        inp_src = lambda j, ch: inp_v[:, offs[j]:offs[j] + ch]
        tgt_src = lambda j, ch: tgt_v[:, offs[j]:offs[j] + ch]

    with tile.TileContext(nc) as tc:
        with (
            tc.tile_pool(name="eps", bufs=1) as eps_pool,
            tc.tile_pool(name="io", bufs=io_bufs) as io_pool,
            tc.tile_pool(name="scratch", bufs=scratch_bufs) as scratch_pool,
            tc.tile_pool(name="acc", bufs=1) as acc_pool,
        ):
            # EPS bias for the ACT Ln; Tile tracks the memset->ACT dep so
            # it overlaps the first DMAs (no extra all-engine barrier)
            if compute != "none":
                eps_t = eps_pool.tile([P, 1], mybir.dt.float32)
                nc.gpsimd.memset(eps_t[:], EPS)
            if warmup_dma:
                wt = eps_pool.tile([P, 1], mybir.dt.float32, tag="warm")
                nc.sync.dma_start(wt[:], inp_src(0, max_ch)[:, 0:1])
                nc.vector.tensor_copy(wt[:], wt[:])  # keep a reader

            acc = None
            if compute == "full":
                acc = acc_pool.tile([P, n_out], mybir.dt.float32)
            last_tt = None
            for it in range(nch * repeat):
                j = it % nch
                ch = ch_schedule[j]
                if alt_dma == "cross":
                    # both HWDGE rings busy every chunk, each carrying one
                    # tensor, swapping per chunk so neither ring owns a
                    # tensor's full stream
                    dma = nc.sync if it % 2 == 0 else nc.scalar
                elif alt_dma == "crossg":
                    # like cross, but the second issue stream is gpsimd
                    # SWDGE (otherwise idle) instead of the ACT/scalar
                    # queue, which also carries the Ln activations
                    dma = nc.sync if it % 2 == 0 else nc.gpsimd
                elif alt_dma == "tri":
                    # three issue streams (2 HWDGE rings + gpsimd SWDGE):
                    # probes whether per-transfer issue gaps on a ring cost
                    # steady-state bandwidth
                    dma = (nc.sync, nc.scalar, nc.gpsimd)[it % 3]
                else:
                    dma = nc.scalar if (alt_dma and it % 2) else nc.sync
                if alt_dma == "cross":
                    dma_inp = nc.scalar if it % 2 == 0 else nc.sync
                elif alt_dma == "crossg":
                    dma_inp = nc.gpsimd if it % 2 == 0 else nc.sync
                elif alt_dma == "tri":
                    dma_inp = (nc.scalar, nc.gpsimd, nc.sync)[it % 3]
                elif split_rings == "gpsimd":
                    dma_inp = nc.gpsimd
                elif split_rings:
                    dma_inp = nc.scalar
                else:
                    dma_inp = dma
                if tail_split > 1 and j == nch - 1:
                    # Tail drain shortener: the pass's LAST tgt AND inp
                    # chunks each land as tail_split small transfers (on
                    # their own rings under split_rings, where both rings
                    # finish together), each slice chased by ACT then DVE,
                    # so the serial drain after the final DMA byte is one
                    # sub-slice (ch/tail_split) instead of a whole chunk.
                    # Same total bytes; the extra transfers sit at the
                    # stream end.  DMA-issue instructions for the scalar
                    # ring are emitted BEFORE the Ln slices so they are not
                    # queued behind ACT compute on the shared sequencer.
                    # Descending pieces: same transfer count as equal
                    # pieces, but the FINAL piece (which alone sets the
                    # serial drain) is ch/8 instead of ch/4.
                    if tail_split == 4 and ch % 16 == 0:
                        psched = [ch * 5 // 16, ch * 5 // 16,
                                  ch * 4 // 16, ch * 2 // 16]
                    else:
                        psched = [ch // tail_split] * tail_split
                    poffs = [offs[j]]
                    for c in psched:
                        poffs.append(poffs[-1] + c)
                    prod = scratch_pool.tile([P, max_ch], mybir.dt.float32)
                    tqs, tps = [], []
                    for s in range(tail_split):
                        tq = io_pool.tile([P, psched[s]], mybir.dt.float32,
                                          tag=f"tailq{s}", bufs=1,
                                          name=f"tq{s}")
                        dma.dma_start(tq[:], tgt_v[:, poffs[s]:poffs[s + 1]])
                        tqs.append(tq)
                    for s in range(tail_split):
                        tp = io_pool.tile([P, psched[s]], mybir.dt.float32,
                                          tag=f"tail{s}", bufs=1,
                                          name=f"tp{s}")
                        dma_inp.dma_start(tp[:],
                                          inp_v[:, poffs[s]:poffs[s + 1]])
                        tps.append(tp)
                    base = poffs[0]
                    for s in range(tail_split):
                        sl = slice(poffs[s] - base, poffs[s + 1] - base)
                        nc.scalar.activation(prod[:, sl], tqs[s][:],
                                             mybir.ActivationFunctionType.Ln,
                                             bias=eps_t[:])
                        nc.vector.tensor_tensor(prod[:, sl], tps[s][:],
                                                prod[:, sl],
                                                mybir.AluOpType.mult)
                        nc.vector.tensor_reduce(acc[:, j + s:j + s + 1],
                                                prod[:, sl],
                                                mybir.AxisListType.X,
                                                mybir.AluOpType.add)
                    last_tt = tqs[-1]
                    continue
                # target first: ACT only needs tgt, so it can start while
                # input is still in flight
                tt = io_pool.tile([P, max_ch], mybir.dt.float32, tag="tgt")
                dma.dma_start(tt[:, :ch], tgt_src(j, ch))
                ti = io_pool.tile([P, max_ch], mybir.dt.float32, tag="inp")
                dma_inp.dma_start(ti[:, :ch], inp_src(j, ch))
                last_tt = tt
                if compute == "none":
                    continue
                if act_oop:
                    # log into scratch: tt's buffer frees right after ACT
                    # reads it, giving tgt DMAs one more stage of lead time
                    prod = scratch_pool.tile([P, max_ch], mybir.dt.float32)
                    nc.scalar.activation(prod[:, :ch], tt[:, :ch],
                                         mybir.ActivationFunctionType.Ln,
                                         bias=eps_t[:])
                    if compute == "act":
                        continue
                    nc.vector.tensor_tensor(prod[:, :ch], ti[:, :ch],
                                            prod[:, :ch],
                                            mybir.AluOpType.mult)
                    nc.vector.tensor_reduce(acc[:, j:j + 1], prod[:, :ch],
                                            mybir.AxisListType.X,
                                            mybir.AluOpType.add)
                    continue
                # tt = log(tt + EPS), in place on the ACT engine
                nc.scalar.activation(tt[:, :ch], tt[:, :ch],
                                     mybir.ActivationFunctionType.Ln,
                                     bias=eps_t[:])
                if compute == "act":
                    continue
                # acc[:, j] = sum_free(ti * tt)
                # (TensorTensorReduce would fuse these, but it crashes the
                # device on this runtime build -- use 2 DVE ops instead)
                if inplace_mult:
                    prod = ti
                else:
                    prod = scratch_pool.tile([P, max_ch], mybir.dt.float32)
                nc.vector.tensor_tensor(prod[:, :ch], ti[:, :ch], tt[:, :ch],
                                        mybir.AluOpType.mult)
                nc.vector.tensor_reduce(acc[:, j:j + 1], prod[:, :ch],
                                        mybir.AxisListType.X,
                                        mybir.AluOpType.add)
            if compute == "full":
                nc.sync.dma_start(out[:], acc[:])
            else:  # timing probes: output is garbage, deps only on last tile
                nc.sync.dma_start(out[:], last_tt[:, :n_out])
    nc.compile()
    return nc


def shard_inputs(inp: np.ndarray, tgt: np.ndarray) -> list[dict]:
    return [
        {
            "input": np.ascontiguousarray(inp[i * ROWS:(i + 1) * ROWS]),
            "target": np.ascontiguousarray(tgt[i * ROWS:(i + 1) * ROWS]),
        }
        for i in range(NCORES)
    ]


_PJRT_EXEC = None


def _make_pjrt_exec(nc):
    """Reusable jitted SPMD executable (mirror of bass2jax.run_bass_via_pjrt's
    multi-core path).  run_bass_kernel_spmd re-creates and re-jits this
    wrapper on every call; caching it makes repeated kernel() calls skip
    the re-trace/re-lower (and, without a persistent jax cache, a full
    neuronxcc recompile) entirely."""
    import jax
    from jax.experimental.shard_map import shard_map
    from jax.sharding import Mesh, NamedSharding, PartitionSpec
    from concourse import bass2jax

    bass2jax.install_neuronx_cc_hook()
    assert nc.dbg_addr is None  # debug=False build
    partition_name = (nc.partition_id_tensor.name
                      if nc.partition_id_tensor else None)
    in_names, out_names, out_avals, zero_shapes = [], [], [], []
    for alloc in nc.m.functions[0].allocations:
        if not isinstance(alloc, mybir.MemoryLocationSet):
            continue
        name = alloc.memorylocations[0].name
        if alloc.kind == "ExternalInput":
            if name != partition_name:
                in_names.append(name)
        elif alloc.kind == "ExternalOutput":
            shape = tuple(alloc.tensor_shape)
            dtype = mybir.dt.np(alloc.dtype)
            out_names.append(name)
            out_avals.append(jax.core.ShapedArray(shape, dtype))
            zero_shapes.append(((NCORES * shape[0], *shape[1:]), dtype))
    n_outs = len(out_avals)
    all_in = list(in_names) + out_names
    if partition_name is not None:
        all_in.append(partition_name)
    nin = len(all_in) - (1 if partition_name else 0)
    donate = tuple(range(len(in_names), len(in_names) + n_outs))

    def _body(*args):
        operands = list(args)
        if partition_name is not None:
            operands.append(bass2jax.partition_id_tensor())
        return tuple(bass2jax._bass_exec_p.bind(
            *operands,
            out_avals=tuple(out_avals),
            in_names=tuple(all_in),
            out_names=tuple(out_names),
            lowering_input_output_aliases=(),
            sim_require_finite=True,
            sim_require_nnan=True,
            nc=nc,
        ))

    mesh = Mesh(np.asarray(jax.devices()[:NCORES]), ("core",))
    fn = jax.jit(
        shard_map(_body, mesh=mesh,
                  in_specs=(PartitionSpec("core"),) * nin,
                  out_specs=(PartitionSpec("core"),) * n_outs,
                  check_rep=False),
        donate_argnums=donate, keep_unused=True)
    sharding = NamedSharding(mesh, PartitionSpec("core"))
    return fn, in_names, zero_shapes, sharding


def combine(results: list[dict]) -> np.ndarray:
    total = 0.0
    for r in results:
        total += float(np.sum(np.asarray(r["out"], dtype=np.float64)))
    return np.array([-total / B], dtype=np.float32)


def kernel(**inputs: np.ndarray) -> np.ndarray:
    global _NC_CACHE, _PJRT_EXEC
    inp = np.ascontiguousarray(np.asarray(inputs["input"], dtype=np.float32))
    tgt = np.ascontiguousarray(np.asarray(inputs["target"], dtype=np.float32))
    assert inp.shape == (B, C) and tgt.shape == (B, C)

    if _NC_CACHE is None:
        _NC_CACHE = build_nc()
    nc = _NC_CACHE

    from concourse.bass_utils import axon_active
    if not axon_active():
        # native /dev/neuron* environment: the NrtSession path
        res = run_bass_kernel_spmd(nc, shard_inputs(inp, tgt),
                                   list(range(NCORES)))
        return combine(res.results)

    # axon: one cached jitted executable for all calls.  Each core's shard
    # is a contiguous row block, so the concatenated global input IS the
    # full array — device_put with core sharding, no host-side slicing.
    import jax
    if _PJRT_EXEC is None:
        _PJRT_EXEC = _make_pjrt_exec(nc)
    fn, in_names, zero_shapes, sharding = _PJRT_EXEC
    full = {"input": inp, "target": tgt}
    args = [jax.device_put(full[n], sharding) for n in in_names]
    zeros = [np.zeros(s, d) for s, d in zero_shapes]
    outs = fn(*args, *zeros)
    out = np.asarray(outs[0], dtype=np.float64)  # (NCORES*P, n_out)
    return np.array([-out.sum() / B], dtype=np.float32)

